# revision 33
# baseline (speedup 1.0000x reference)
"""BKT (Bayesian Knowledge Tracing) forward pass on Trainium2, 8 NeuronCores.

The reference's chunked 32-trajectory scan is a 2-state HMM forward pass.
Per (sequence, t):  alpha' = alpha @ (diag(o_t) @ Tr), with o_s(t) =
P(obs_t | L=s) and Tr the 2x2 BKT transition matrix. The output is the
log-softmax over [P(incorrect), P(correct)], i.e. per-t it only depends on
the normalized alpha — every intermediate may carry an arbitrary per-t scale,
which this kernel exploits aggressively.

v3 design (engine-balanced, fp16 2x-mode core, software-pipelined 2 deep):
  - Host sends sign-flipped logits zpk (fp16) so one ACT Sigmoid call gives
    o_s(t) = P(observed outcome | s); the device emits [log(1-r), log r] with
    r = P(observed)/den and the host swaps slots where corr==0 (marshaling).
  - Per-step matrices W = o x (2*Tr) in fp16. Chunk products = two half-chunk
    products of 5 (fp16, range-safe [2^-15, 2^4]), joined in f32 on Pool.
  - Chunk matrices are sum-normalized (one DVE reciprocal per segment), which
    keeps the 50-step f32 serial chunk recursion on Pool bounded (max drift
    2^79 on this data) with NO in-loop renormalization or division.
  - Within-chunk recovery in fp16 from reciprocal-normalized chunk starts,
    restarting mid-chunk (per-chunk/per-half scales cancel in r).
  - Predictions: qp in f32 (DVE), pair-sums on Pool, three ACT Ln calls,
    final log-softmax subtractions on DVE in fp16.
  - Pipeline skew: segment s's fold (DVE) overlaps segment s-1's back half;
    Pool stream ordered [join(s) | preds(s-1) | serial(s)] so the in-order
    Pool queue never blocks on a not-yet-ready stage.
All hot-loop DVE traffic is 2-byte packed (0.52 ns/elem 2x mode); DRAM
arrays are host-packed so every DMA descriptor is a 6.4KB contiguous run.

Sharding: pure data-parallel over batch (2048 sequences/core as 128
partitions x 16 groups); parameter tables gathered on host.
"""

import numpy as np

import concourse.bass as bass
import concourse.bacc as bacc
import concourse.tile as tile
import concourse.mybir as mybir

F32 = mybir.dt.float32
F16 = mybir.dt.float16
AF = mybir.ActivationFunctionType
OP = mybir.AluOpType

P = 128          # partitions
N_CORES = 8
GAMMA = 2.0      # per-step scale baked into Tr: keeps fp16 products ~1


def emit_bkt(nc, G, T, K, SEG):
    """Emit the BKT kernel for one core. Sequences = P*G, free layout (.., g).

    DRAM tensors:
      zpk:  (P, T, 2, G) f16  sign-flipped [guess, slip] logits:
            zpk[..0] = (2c-1)*lg, zpk[..1] = -(2c-1)*ls
      dyn:  (P, 3, G) f32     [logit_pL, logit_pF, logit_pI0]
      out:  (P, T, 2, G) f16  [log(1-r), log r], r = P(observed outcome)
    """
    assert T % SEG == 0 and SEG % K == 0 and K % 2 == 0
    NSEG = T // SEG
    CS = SEG // K          # chunks per segment
    CT = T // K            # total chunks
    H = K // 2             # half-chunk length
    C2 = 2 * CS            # half-chunks per segment

    zpk_d = nc.dram_tensor("zpk", [P, T, 2, G], F16, kind="ExternalInput")
    dyn_d = nc.dram_tensor("dyn", [P, 3, G], F32, kind="ExternalInput")
    out_d = nc.dram_tensor("out", [P, T, 2, G], F16, kind="ExternalOutput")

    with tile.TileContext(nc) as tc:
        with (
            tc.tile_pool(name="singles", bufs=1) as singles,
            tc.tile_pool(name="dbuf", bufs=2) as dbuf,
            tc.tile_pool(name="scratch", bufs=1) as scratch,
        ):
            # ---- per-sequence constants (zpk slice 0 DMA'd first) ----
            zpk0 = dbuf.tile([P, SEG, 2, G], F16, tag="zpk")
            nc.sync.dma_start(zpk0[:, : SEG // 4], zpk_d[:, : SEG // 4])
            dyn_t = singles.tile([P, 3, G], F32)
            nc.sync.dma_start(dyn_t[:], dyn_d[:])
            Ttmp = singles.tile([P, 2, 2, G], F32)   # Tr[s][s'][g]
            nc.scalar.activation(Ttmp[:, 0, 0], dyn_t[:, 0], AF.Sigmoid, scale=-1.0)
            nc.scalar.activation(Ttmp[:, 0, 1], dyn_t[:, 0], AF.Sigmoid)
            nc.scalar.activation(Ttmp[:, 1, 0], dyn_t[:, 1], AF.Sigmoid)
            nc.scalar.activation(Ttmp[:, 1, 1], dyn_t[:, 1], AF.Sigmoid, scale=-1.0)
            Tp = singles.tile([P, 2, 2, G], F16)     # gamma * Tr
            nc.scalar.mul(Tp[:], Ttmp[:], GAMMA)

            # chunk-start alphas (f32), all chunks + final carry
            starts = singles.tile([P, CT + 1, 2, G], F32)
            nc.scalar.activation(starts[:, 0, 0], dyn_t[:, 2], AF.Sigmoid, scale=-1.0)
            nc.scalar.activation(starts[:, 0, 1], dyn_t[:, 2], AF.Sigmoid)

            obs = {}       # seg -> op tile (sigmoid outputs)
            mats = {}      # seg -> (Wp, Ah) tiles live into the back half
            tiles = {}     # shared tiles for range-split back phases
            preds = {}     # seg -> (pred, den) awaiting the ratio tail
            fin = {}       # seg -> out tile awaiting store

            def phase_a(seg, nsplit=1):
                s0 = seg * SEG
                zpk = zpk0 if seg == 0 else dbuf.tile(
                    [P, SEG, 2, G], F16, tag="zpk")
                op_t = dbuf.tile([P, SEG, 2, G], F16, tag="op")
                bounds = [SEG * h // nsplit for h in range(nsplit + 1)]
                for a, b in zip(bounds, bounds[1:]):
                    if not (seg == 0 and a == 0):
                        nc.sync.dma_start(zpk[:, a:b], zpk_d[:, s0 + a : s0 + b])
                    nc.scalar.activation(op_t[:, a:b], zpk[:, a:b], AF.Sigmoid)
                obs[seg] = op_t

            def front(seg):
                """W build + half-chunk fold (DVE) + f32 join (Pool) +
                A-normalization (DVE) staged for the Pool serial chain."""
                # W[t][s][s'][g] = o[t][s][g] * (gamma Tr)[s][s'][g]   (fp16)
                op_t = obs[seg]
                Wp = dbuf.tile([P, SEG, 2, 2, G], F16, tag="Wp")
                nw = 4 if seg == 0 else 1
                wb = [SEG * h // nw for h in range(nw + 1)]
                for a, b in zip(wb, wb[1:]):
                    for s in range(2):   # split keeps reads within 3 AP dims
                        nc.vector.tensor_tensor(
                            Wp[:, a:b, s],
                            op_t[:, a:b, s].unsqueeze(2).broadcast_to(
                                (P, b - a, 2, G)),
                            Tp[:, s].unsqueeze(1).broadcast_to((P, b - a, 2, G)),
                            OP.mult,
                        )
                Wh = Wp[:].rearrange("p (c h) s u g -> p c h s u g", h=H)

                # half-chunk products Ah[c2][i][s'][g] (fp16); step 1 reads
                # W0 x W1 directly ((i,m) split keeps APs legal, no init copy)
                Ah = dbuf.tile([P, C2, 2, 2, G], F16, tag="Ah")
                TMh = dbuf.tile([P, C2, 2, 2, 2, G], F16, tag="TMh")
                for i in range(2):
                    for m in range(2):
                        nc.vector.tensor_tensor(
                            TMh[:, :, i, m],
                            Wh[:, :, 0, i, m].unsqueeze(2).broadcast_to(
                                (P, C2, 2, G)),
                            Wh[:, :, 1, m],
                            OP.mult,
                        )
                nc.vector.tensor_tensor(
                    Ah[:], TMh[:, :, :, 0], TMh[:, :, :, 1], OP.add
                )
                for j in range(2, H):
                    nc.vector.tensor_tensor(
                        TMh[:],
                        Ah[:].unsqueeze(4).broadcast_to((P, C2, 2, 2, 2, G)),
                        Wh[:, :, j].unsqueeze(2).broadcast_to((P, C2, 2, 2, 2, G)),
                        OP.mult,
                    )
                    nc.vector.tensor_tensor(
                        Ah[:], TMh[:, :, :, 0], TMh[:, :, :, 1], OP.add
                    )
                mats[seg] = (Wp, Ah)

                # join halves -> full chunk products A (f32) on Pool
                Ahv = Ah[:].rearrange("p (c h) i u g -> p c h i u g", h=2)
                TM2 = scratch.tile([P, CS, 2, 2, 2, G], F32, tag="TM2")
                for i in range(2):   # split keeps reads within 3 AP dims
                    for m in range(2):
                        nc.vector.tensor_tensor(
                            TM2[:, :, i, m],
                            Ahv[:, :, 0, i, m].unsqueeze(2).broadcast_to(
                                (P, CS, 2, G)),
                            Ahv[:, :, 1, m],
                            OP.mult,
                        )
                A = dbuf.tile([P, CS, 2, 2, G], F32, tag="A")
                nc.vector.tensor_tensor(
                    A[:], TM2[:, :, :, 0], TM2[:, :, :, 1], OP.add
                )
                return A

            def a_norm(seg, A):
                """Sum-normalize chunk matrices (DVE) so the serial chain
                needs no in-loop renorm; any per-chunk scale cancels."""
                uA = scratch.tile([P, CS, 2, G], F32, tag="uA")
                nc.vector.tensor_tensor(uA[:], A[:, :, 0], A[:, :, 1], OP.add)
                tA = scratch.tile([P, CS, G], F32, tag="tA")
                nc.vector.tensor_tensor(tA[:], uA[:, :, 0], uA[:, :, 1], OP.add)
                nc.vector.reciprocal_approx_fast(tA[:], tA[:])
                Af = A[:].rearrange("p c i u g -> p c (i u) g")
                nc.vector.tensor_tensor(
                    Af, Af,
                    tA[:].unsqueeze(2).broadcast_to((P, CS, 4, G)),
                    OP.mult,
                )

            def serial(seg, A):
                """50-step chunk recursion on Pool, f32, no renorm."""
                c0 = seg * CS
                sv = scratch.tile([P, 2, 2, G], F32, tag="sv")
                for cl in range(CS):
                    cg = c0 + cl
                    nc.gpsimd.tensor_tensor(
                        sv[:],
                        starts[:, cg].unsqueeze(2).broadcast_to((P, 2, 2, G)),
                        A[:, cl],
                        OP.mult,
                    )
                    nc.gpsimd.tensor_tensor(
                        starts[:, cg + 1], sv[:, 0], sv[:, 1], OP.add
                    )

            def back(seg, ca=0, cb=CS):
                """Recovery + predictions for chunks [ca, cb) of segment seg.

                rec2 layout (K outermost) keeps every recovery read within
                3 AP dims ((c,m) merge) -> one instruction per step; the
                host undoes the (k, c) interleave when unsharding. The out
                tile is chunk-major so the last segment can drain in halves.
                """
                c0 = seg * CS
                op_t = obs[seg]
                Wp, Ah = mats[seg]
                if cb == CS:
                    obs.pop(seg)
                    mats.pop(seg)
                Wc = Wp[:].rearrange("p (c k) s u g -> p c k s u g", k=K)
                Ahv = Ah[:].rearrange("p (c h) i u g -> p c h i u g", h=2)
                if ca == 0:
                    rec2 = dbuf.tile([P, K, CS, 2, G], F16, tag="rec2")
                    qp = scratch.tile([P, K, CS, 2, G], F32, tag="qp")
                    den = scratch.tile([P, K, CS, G], F32, tag="den")
                    pred = scratch.tile([P, K, CS, G], F32, tag="pred")
                    tiles[seg] = (rec2, qp, den, pred)
                rec2, qp, den, pred = tiles[seg]

                # normalized fp16 chunk starts -> rec2[., 0]; two halves so
                # the first can start before the serial chain finishes
                if ca == 0:
                    ssc = scratch.tile([P, CS, G], F32, tag="ssc")
                    tiles[seg, "ssc"] = ssc
                ssc = tiles[seg, "ssc"]
                nh = max(1, (cb - ca) // (CS // 2))
                sb = [ca + (cb - ca) * h // nh for h in range(nh + 1)]
                for a, b in zip(sb, sb[1:]):
                    stseg = starts[:, c0 + a : c0 + b]
                    n = b - a
                    nc.vector.tensor_tensor(
                        ssc[:, a:b], stseg[:, :, 0], stseg[:, :, 1], OP.add
                    )
                    nc.vector.reciprocal_approx_fast(ssc[:, a:b], ssc[:, a:b])
                    nc.vector.tensor_tensor(
                        rec2[:, 0, a:b], stseg,
                        ssc[:, a:b].unsqueeze(2).broadcast_to((P, n, 2, G)),
                        OP.mult,
                    )

                # mid-chunk restart: S5 = stn16 . Ah_even, renormalized
                nct = cb - ca
                TM5 = scratch.tile([P, CS, 2, 2, G], F16, tag="TM5")
                for i in range(2):
                    nc.vector.tensor_tensor(
                        TM5[:, ca:cb, i],
                        rec2[:, 0, ca:cb, i].unsqueeze(2).broadcast_to(
                            (P, nct, 2, G)),
                        Ahv[:, ca:cb, 0, i],
                        OP.mult,
                    )
                S5 = scratch.tile([P, CS, 2, G], F16, tag="S5")
                nc.vector.tensor_tensor(
                    S5[:, ca:cb], TM5[:, ca:cb, 0], TM5[:, ca:cb, 1], OP.add
                )
                ss5 = scratch.tile([P, CS, G], F32, tag="ss5")
                nc.vector.tensor_tensor(
                    ss5[:, ca:cb], S5[:, ca:cb, 0], S5[:, ca:cb, 1], OP.add
                )
                nc.vector.reciprocal_approx_fast(ss5[:, ca:cb], ss5[:, ca:cb])
                nc.vector.tensor_tensor(
                    rec2[:, H, ca:cb], S5[:, ca:cb],
                    ss5[:, ca:cb].unsqueeze(2).broadcast_to((P, nct, 2, G)),
                    OP.mult,
                )

                # within-chunk recovery (fp16), both halves, 2 instrs/step
                RR = scratch.tile([P, CS, 2, 2, G], F16, tag="RR")
                for j in list(range(1, H)) + list(range(H + 1, K)):
                    nc.vector.tensor_tensor(
                        RR[:, ca:cb],
                        rec2[:, j - 1, ca:cb].unsqueeze(3).broadcast_to(
                            (P, nct, 2, 2, G)),
                        Wc[:, ca:cb, j - 1],
                        OP.mult,
                    )
                    nc.vector.tensor_tensor(
                        rec2[:, j, ca:cb], RR[:, ca:cb, 0], RR[:, ca:cb, 1],
                        OP.add
                    )

                # predictions: den + reciprocal stay on DVE (no cross-
                # engine wait); qp1/pred/ratio on Pool; Ln on ACT.
                opv = op_t[:].rearrange("p (c k) s g -> p k c s g", k=K)
                tail_eng = nc.vector if seg == NSEG - 1 else nc.gpsimd
                tail_eng.tensor_tensor(
                    qp[:, :, ca:cb, 1], rec2[:, :, ca:cb, 1],
                    opv[:, :, ca:cb, 1], OP.mult
                )
                nc.vector.tensor_tensor(
                    den[:, :, ca:cb], rec2[:, :, ca:cb, 0],
                    rec2[:, :, ca:cb, 1], OP.add
                )
                den_f = den[:, :, ca:cb].rearrange("p k c g -> p k (c g)")
                nc.vector.reciprocal_approx_fast(den_f, den_f)
                nc.vector.tensor_tensor(
                    qp[:, :, ca:cb, 0], rec2[:, :, ca:cb, 0],
                    opv[:, :, ca:cb, 0], OP.mult
                )
                tail_eng.tensor_tensor(
                    pred[:, :, ca:cb], qp[:, :, ca:cb, 0], qp[:, :, ca:cb, 1],
                    OP.add
                )
                preds[seg, ca] = (pred, den)

            def back2(seg, ca=0, cb=CS):
                """Prediction tail: ratio (Pool) + Ln (ACT)."""
                pred, den = preds.pop((seg, ca))
                if ca == 0:
                    rr_t = scratch.tile([P, K, CS, G], F32, tag="rr_t")
                    out_t = dbuf.tile([P, CS, K, 2, G], F16, tag="out")
                    tiles[seg, "out"] = (rr_t, out_t)
                rr_t, out_t = tiles[seg, "out"]
                t_eng = nc.vector if seg == NSEG - 1 else nc.gpsimd
                t_eng.tensor_tensor(
                    rr_t[:, :, ca:cb], pred[:, :, ca:cb], den[:, :, ca:cb],
                    OP.mult
                )
                ov = out_t[:].rearrange("p c k u g -> p k c u g")
                nc.scalar.activation(ov[:, :, ca:cb, 1], rr_t[:, :, ca:cb], AF.Ln)
                nc.scalar.activation(
                    ov[:, :, ca:cb, 0], rr_t[:, :, ca:cb], AF.Ln,
                    scale=-1.0, bias=1.0
                )
                fin[seg, ca] = out_t

            def finalize(seg, ca=0, cb=CS):
                out_t = fin.pop((seg, ca))
                s0 = seg * SEG
                nc.sync.dma_start(
                    out_d[:, s0 + ca * K : s0 + cb * K], out_t[:, ca:cb]
                )

            # ---- software pipeline, 2 segments deep ----
            # Per iteration the Pool stream is [qp1/den/pred(s-1) | join(s) |
            # serial(s)]: every stage is data-ready when the in-order queue
            # reaches it, so Pool work spreads across the whole iteration.
            phase_a(0, nsplit=4)
            phase_a(1)
            for seg in range(NSEG):
                if seg >= 1:
                    back(seg - 1)
                A = front(seg)
                a_norm(seg, A)
                if seg >= 1:
                    back2(seg - 1)
                serial(seg, A)
                if seg >= 1:
                    finalize(seg - 1)
                if seg + 2 < NSEG:
                    phase_a(seg + 2)
            hc = CS // 2
            back(NSEG - 1, 0, hc)
            back2(NSEG - 1, 0, hc)
            back(NSEG - 1, hc, CS)
            finalize(NSEG - 1, 0, hc)
            back2(NSEG - 1, hc, CS)
            finalize(NSEG - 1, hc, CS)

    return nc


# ------------------------------------------------------------------
# Host-side full-problem wrapper
# ------------------------------------------------------------------

_B, _T, _K, _SEG = 16384, 500, 10, 100
_G = _B // (P * N_CORES)   # 16 groups per core

_cached = {}


def _build():
    if "nc" not in _cached:
        nc = bacc.Bacc(None, target_bir_lowering=False)
        emit_bkt(nc, G=_G, T=_T, K=_K, SEG=_SEG)
        nc.compile()
        _cached["nc"] = nc
    return _cached["nc"]


def _shard(arr, core):
    """(B,...) -> this core's (P, ..., G) permuted view, seq = g*128 + p."""
    rows = arr[core * P * _G : (core + 1) * P * _G]
    r = rows.reshape(_G, P, *arr.shape[1:])
    order = (1,) + tuple(range(2, r.ndim)) + (0,)
    return np.ascontiguousarray(r.transpose(order))


def kernel(corr, kc, problem, dynamics_logits_table, obs_logits_kc,
           obs_logits_problem, fastbkt_n):
    from concourse.bass_utils import run_bass_kernel_spmd

    corr = np.asarray(corr, dtype=np.float32)
    kc = np.asarray(kc).astype(np.int64)
    problem = np.asarray(problem).astype(np.int64)
    dyn_table = np.asarray(dynamics_logits_table, dtype=np.float32)
    obs_kc = np.asarray(obs_logits_kc, dtype=np.float32)
    obs_prob = np.asarray(obs_logits_problem, dtype=np.float32)

    B, T = corr.shape
    assert B == _B and T == _T, (B, T)

    # host gathers + sign-flip (traffic-neutral input marshaling)
    lls = obs_kc[kc][:, None, :] + obs_prob[problem]       # (B, T, 2)
    sgn = (corr * 2.0 - 1.0).astype(np.float32)            # (B, T)
    zpk = np.empty((B, T, 2), np.float16)
    zpk[:, :, 0] = sgn * lls[:, :, 0]
    zpk[:, :, 1] = -sgn * lls[:, :, 1]
    dyn = dyn_table[kc]                                    # (B, 3)

    nc = _build()
    in_maps = []
    for core in range(N_CORES):
        in_maps.append({
            "zpk": _shard(zpk, core),
            "dyn": _shard(dyn, core),
        })

    res = run_bass_kernel_spmd(
        nc, in_maps, core_ids=list(range(N_CORES)), **_cached.get("run_kwargs", {})
    )
    _cached["last_results"] = res

    # unshard + slot swap: device slot1 = log P(observed), slot0 = log P(other)
    # device time order within a segment is (c, k): t = seg*SEG + c*K + k
    NSEG, CS = _T // _SEG, _SEG // _K
    dev = np.empty((B, T, 2), np.float32)
    for core in range(N_CORES):
        o = res.results[core]["out"].astype(np.float32)    # (P, T, 2, G)
        o = o.reshape(P, NSEG, CS, _K, 2, _G)
        rows = o.transpose(5, 0, 1, 2, 3, 4).reshape(P * _G, T, 2)
        dev[core * P * _G : (core + 1) * P * _G] = rows
    c1 = corr > 0.5
    out = np.empty((B, T, 2), np.float32)
    out[:, :, 1] = np.where(c1, dev[:, :, 1], dev[:, :, 0])
    out[:, :, 0] = np.where(c1, dev[:, :, 0], dev[:, :, 1])
    return out


# revision 34
# speedup vs baseline: 1.0157x; 1.0157x over previous
"""BKT (Bayesian Knowledge Tracing) forward pass on Trainium2, 8 NeuronCores.

The reference's chunked 32-trajectory scan is a 2-state HMM forward pass.
Per (sequence, t):  alpha' = alpha @ (diag(o_t) @ Tr), with o_s(t) =
P(obs_t | L=s) and Tr the 2x2 BKT transition matrix. The output is the
log-softmax over [P(incorrect), P(correct)], i.e. per-t it only depends on
the normalized alpha — every intermediate may carry an arbitrary per-t scale,
which this kernel exploits aggressively.

v3 design (engine-balanced, fp16 2x-mode core, software-pipelined 2 deep):
  - Host sends sign-flipped logits zpk (fp16) so one ACT Sigmoid call gives
    o_s(t) = P(observed outcome | s); the device emits [log(1-r), log r] with
    r = P(observed)/den and the host swaps slots where corr==0 (marshaling).
  - Per-step matrices W = o x (2*Tr) in fp16. Chunk products = two half-chunk
    products of 5 (fp16, range-safe [2^-15, 2^4]), joined in f32 on Pool.
  - Chunk matrices are sum-normalized (one DVE reciprocal per segment), which
    keeps the 50-step f32 serial chunk recursion on Pool bounded (max drift
    2^79 on this data) with NO in-loop renormalization or division.
  - Within-chunk recovery in fp16 from reciprocal-normalized chunk starts,
    restarting mid-chunk (per-chunk/per-half scales cancel in r).
  - Predictions: qp in f32 (DVE), pair-sums on Pool, three ACT Ln calls,
    final log-softmax subtractions on DVE in fp16.
  - Pipeline skew: segment s's fold (DVE) overlaps segment s-1's back half;
    Pool stream ordered [join(s) | preds(s-1) | serial(s)] so the in-order
    Pool queue never blocks on a not-yet-ready stage.
All hot-loop DVE traffic is 2-byte packed (0.52 ns/elem 2x mode); DRAM
arrays are host-packed so every DMA descriptor is a 6.4KB contiguous run.

Sharding: pure data-parallel over batch (2048 sequences/core as 128
partitions x 16 groups); parameter tables gathered on host.
"""

import numpy as np

import concourse.bass as bass
import concourse.bacc as bacc
import concourse.tile as tile
import concourse.mybir as mybir

F32 = mybir.dt.float32
F16 = mybir.dt.float16
AF = mybir.ActivationFunctionType
OP = mybir.AluOpType

P = 128          # partitions
N_CORES = 8
GAMMA = 2.0      # per-step scale baked into Tr: keeps fp16 products ~1


def emit_bkt(nc, G, T, K, SEG):
    """Emit the BKT kernel for one core. Sequences = P*G, free layout (.., g).

    DRAM tensors:
      zpk:  (P, T, 2, G) f16  sign-flipped [guess, slip] logits:
            zpk[..0] = (2c-1)*lg, zpk[..1] = -(2c-1)*ls
      dyn:  (P, 3, G) f32     [logit_pL, logit_pF, logit_pI0]
      out:  (P, T, 2, G) f16  [log(1-r), log r], r = P(observed outcome)
    """
    assert T % SEG == 0 and SEG % K == 0 and K % 2 == 0
    NSEG = T // SEG
    CS = SEG // K          # chunks per segment
    CT = T // K            # total chunks
    H = K // 2             # half-chunk length
    C2 = 2 * CS            # half-chunks per segment

    zpk_d = nc.dram_tensor("zpk", [P, T, 2, G], F16, kind="ExternalInput")
    dyn_d = nc.dram_tensor("dyn", [P, 3, G], F32, kind="ExternalInput")
    out_d = nc.dram_tensor("out", [P, T, 2, G], F16, kind="ExternalOutput")

    with tile.TileContext(nc) as tc:
        with (
            tc.tile_pool(name="singles", bufs=1) as singles,
            tc.tile_pool(name="dbuf", bufs=2) as dbuf,
            tc.tile_pool(name="scratch", bufs=1) as scratch,
        ):
            # ---- per-sequence constants (zpk slice 0 DMA'd first) ----
            zpk0 = dbuf.tile([P, SEG, 2, G], F16, tag="zpk")
            nc.sync.dma_start(zpk0[:, : SEG // 4], zpk_d[:, : SEG // 4])
            dyn_t = singles.tile([P, 3, G], F32)
            nc.sync.dma_start(dyn_t[:], dyn_d[:])
            Ttmp = singles.tile([P, 2, 2, G], F32)   # Tr[s][s'][g]
            nc.scalar.activation(Ttmp[:, 0, 0], dyn_t[:, 0], AF.Sigmoid, scale=-1.0)
            nc.scalar.activation(Ttmp[:, 0, 1], dyn_t[:, 0], AF.Sigmoid)
            nc.scalar.activation(Ttmp[:, 1, 0], dyn_t[:, 1], AF.Sigmoid)
            nc.scalar.activation(Ttmp[:, 1, 1], dyn_t[:, 1], AF.Sigmoid, scale=-1.0)
            Tp = singles.tile([P, 2, 2, G], F16)     # gamma * Tr
            nc.scalar.mul(Tp[:], Ttmp[:], GAMMA)

            # chunk-start alphas (f32), all chunks + final carry
            starts = singles.tile([P, CT + 1, 2, G], F32)
            nc.scalar.activation(starts[:, 0, 0], dyn_t[:, 2], AF.Sigmoid, scale=-1.0)
            nc.scalar.activation(starts[:, 0, 1], dyn_t[:, 2], AF.Sigmoid)

            obs = {}       # seg -> op tile (sigmoid outputs)
            mats = {}      # seg -> (Wp, Ah) tiles live into the back half
            tiles = {}     # shared tiles for range-split back phases
            preds = {}     # seg -> (pred, den) awaiting the ratio tail
            fin = {}       # seg -> out tile awaiting store

            def phase_a(seg, nsplit=1):
                s0 = seg * SEG
                zpk = zpk0 if seg == 0 else dbuf.tile(
                    [P, SEG, 2, G], F16, tag="zpk")
                op_t = dbuf.tile([P, SEG, 2, G], F16, tag="op")
                bounds = [SEG * h // nsplit for h in range(nsplit + 1)]
                for a, b in zip(bounds, bounds[1:]):
                    if not (seg == 0 and a == 0):
                        nc.sync.dma_start(zpk[:, a:b], zpk_d[:, s0 + a : s0 + b])
                    nc.scalar.activation(op_t[:, a:b], zpk[:, a:b], AF.Sigmoid)
                obs[seg] = op_t

            def front(seg):
                """W build + half-chunk fold (DVE) + f32 join (Pool) +
                A-normalization (DVE) staged for the Pool serial chain."""
                # W[t][s][s'][g] = o[t][s][g] * (gamma Tr)[s][s'][g]   (fp16)
                op_t = obs[seg]
                Wp = dbuf.tile([P, SEG, 2, 2, G], F16, tag="Wp")
                nw = 4 if seg == 0 else 1
                wb = [SEG * h // nw for h in range(nw + 1)]
                for a, b in zip(wb, wb[1:]):
                    for s in range(2):   # split keeps reads within 3 AP dims
                        nc.vector.tensor_tensor(
                            Wp[:, a:b, s],
                            op_t[:, a:b, s].unsqueeze(2).broadcast_to(
                                (P, b - a, 2, G)),
                            Tp[:, s].unsqueeze(1).broadcast_to((P, b - a, 2, G)),
                            OP.mult,
                        )
                Wh = Wp[:].rearrange("p (c h) s u g -> p c h s u g", h=H)

                # half-chunk products Ah[c2][i][s'][g] (fp16); step 1 reads
                # W0 x W1 directly ((i,m) split keeps APs legal, no init copy)
                Ah = dbuf.tile([P, C2, 2, 2, G], F16, tag="Ah")
                TMh = dbuf.tile([P, C2, 2, 2, 2, G], F16, tag="TMh")
                for i in range(2):
                    for m in range(2):
                        nc.vector.tensor_tensor(
                            TMh[:, :, i, m],
                            Wh[:, :, 0, i, m].unsqueeze(2).broadcast_to(
                                (P, C2, 2, G)),
                            Wh[:, :, 1, m],
                            OP.mult,
                        )
                nc.vector.tensor_tensor(
                    Ah[:], TMh[:, :, :, 0], TMh[:, :, :, 1], OP.add
                )
                for j in range(2, H):
                    nc.vector.tensor_tensor(
                        TMh[:],
                        Ah[:].unsqueeze(4).broadcast_to((P, C2, 2, 2, 2, G)),
                        Wh[:, :, j].unsqueeze(2).broadcast_to((P, C2, 2, 2, 2, G)),
                        OP.mult,
                    )
                    nc.vector.tensor_tensor(
                        Ah[:], TMh[:, :, :, 0], TMh[:, :, :, 1], OP.add
                    )
                mats[seg] = (Wp, Ah)

                # join halves -> full chunk products A (f32) on Pool
                Ahv = Ah[:].rearrange("p (c h) i u g -> p c h i u g", h=2)
                TM2 = scratch.tile([P, CS, 2, 2, 2, G], F32, tag="TM2")
                for i in range(2):   # split keeps reads within 3 AP dims
                    for m in range(2):
                        nc.vector.tensor_tensor(
                            TM2[:, :, i, m],
                            Ahv[:, :, 0, i, m].unsqueeze(2).broadcast_to(
                                (P, CS, 2, G)),
                            Ahv[:, :, 1, m],
                            OP.mult,
                        )
                A = dbuf.tile([P, CS, 2, 2, G], F32, tag="A")
                nc.vector.tensor_tensor(
                    A[:], TM2[:, :, :, 0], TM2[:, :, :, 1], OP.add
                )
                return A

            def a_norm(seg, A):
                """Sum-normalize chunk matrices (DVE) so the serial chain
                needs no in-loop renorm; any per-chunk scale cancels."""
                uA = scratch.tile([P, CS, 2, G], F32, tag="uA")
                nc.vector.tensor_tensor(uA[:], A[:, :, 0], A[:, :, 1], OP.add)
                tA = scratch.tile([P, CS, G], F32, tag="tA")
                nc.vector.tensor_tensor(tA[:], uA[:, :, 0], uA[:, :, 1], OP.add)
                nc.vector.reciprocal_approx_fast(tA[:], tA[:])
                Af = A[:].rearrange("p c i u g -> p c (i u) g")
                nc.vector.tensor_tensor(
                    Af, Af,
                    tA[:].unsqueeze(2).broadcast_to((P, CS, 4, G)),
                    OP.mult,
                )

            def serial(seg, A):
                """50-step chunk recursion on Pool, f32, no renorm."""
                c0 = seg * CS
                sv = scratch.tile([P, 2, 2, G], F32, tag="sv")
                for cl in range(CS):
                    cg = c0 + cl
                    nc.gpsimd.tensor_tensor(
                        sv[:],
                        starts[:, cg].unsqueeze(2).broadcast_to((P, 2, 2, G)),
                        A[:, cl],
                        OP.mult,
                    )
                    nc.gpsimd.tensor_tensor(
                        starts[:, cg + 1], sv[:, 0], sv[:, 1], OP.add
                    )

            def back(seg, ca=0, cb=CS):
                """Recovery + predictions for chunks [ca, cb) of segment seg.

                rec2 layout (K outermost) keeps every recovery read within
                3 AP dims ((c,m) merge) -> one instruction per step; the
                host undoes the (k, c) interleave when unsharding. The out
                tile is chunk-major so the last segment can drain in halves.
                """
                c0 = seg * CS
                op_t = obs[seg]
                Wp, Ah = mats[seg]
                if cb == CS:
                    obs.pop(seg)
                    mats.pop(seg)
                Wc = Wp[:].rearrange("p (c k) s u g -> p c k s u g", k=K)
                Ahv = Ah[:].rearrange("p (c h) i u g -> p c h i u g", h=2)
                if ca == 0:
                    rec2 = dbuf.tile([P, K, CS, 2, G], F16, tag="rec2")
                    qp = scratch.tile([P, K, CS, 2, G], F32, tag="qp")
                    den = scratch.tile([P, K, CS, G], F32, tag="den")
                    pred = scratch.tile([P, K, CS, G], F32, tag="pred")
                    tiles[seg] = (rec2, qp, den, pred)
                rec2, qp, den, pred = tiles[seg]

                # normalized fp16 chunk starts -> rec2[., 0]; two halves so
                # the first can start before the serial chain finishes
                if ca == 0:
                    ssc = scratch.tile([P, CS, G], F32, tag="ssc")
                    tiles[seg, "ssc"] = ssc
                ssc = tiles[seg, "ssc"]
                nh = max(1, (cb - ca) // (CS // 2))
                sb = [ca + (cb - ca) * h // nh for h in range(nh + 1)]
                for a, b in zip(sb, sb[1:]):
                    stseg = starts[:, c0 + a : c0 + b]
                    n = b - a
                    nc.vector.tensor_tensor(
                        ssc[:, a:b], stseg[:, :, 0], stseg[:, :, 1], OP.add
                    )
                    nc.vector.reciprocal_approx_fast(ssc[:, a:b], ssc[:, a:b])
                    nc.vector.tensor_tensor(
                        rec2[:, 0, a:b], stseg,
                        ssc[:, a:b].unsqueeze(2).broadcast_to((P, n, 2, G)),
                        OP.mult,
                    )

                # mid-chunk restart: S5 = stn16 . Ah_even, renormalized
                nct = cb - ca
                TM5 = scratch.tile([P, CS, 2, 2, G], F16, tag="TM5")
                for i in range(2):
                    nc.vector.tensor_tensor(
                        TM5[:, ca:cb, i],
                        rec2[:, 0, ca:cb, i].unsqueeze(2).broadcast_to(
                            (P, nct, 2, G)),
                        Ahv[:, ca:cb, 0, i],
                        OP.mult,
                    )
                S5 = scratch.tile([P, CS, 2, G], F16, tag="S5")
                nc.vector.tensor_tensor(
                    S5[:, ca:cb], TM5[:, ca:cb, 0], TM5[:, ca:cb, 1], OP.add
                )
                ss5 = scratch.tile([P, CS, G], F32, tag="ss5")
                nc.vector.tensor_tensor(
                    ss5[:, ca:cb], S5[:, ca:cb, 0], S5[:, ca:cb, 1], OP.add
                )
                nc.vector.reciprocal_approx_fast(ss5[:, ca:cb], ss5[:, ca:cb])
                nc.vector.tensor_tensor(
                    rec2[:, H, ca:cb], S5[:, ca:cb],
                    ss5[:, ca:cb].unsqueeze(2).broadcast_to((P, nct, 2, G)),
                    OP.mult,
                )

                # within-chunk recovery (fp16), both halves, 2 instrs/step
                RR = scratch.tile([P, CS, 2, 2, G], F16, tag="RR")
                for j in list(range(1, H)) + list(range(H + 1, K)):
                    nc.vector.tensor_tensor(
                        RR[:, ca:cb],
                        rec2[:, j - 1, ca:cb].unsqueeze(3).broadcast_to(
                            (P, nct, 2, 2, G)),
                        Wc[:, ca:cb, j - 1],
                        OP.mult,
                    )
                    nc.vector.tensor_tensor(
                        rec2[:, j, ca:cb], RR[:, ca:cb, 0], RR[:, ca:cb, 1],
                        OP.add
                    )

                # predictions: den + reciprocal stay on DVE (no cross-
                # engine wait); qp1/pred/ratio on Pool; Ln on ACT.
                opv = op_t[:].rearrange("p (c k) s g -> p k c s g", k=K)
                nc.gpsimd.tensor_tensor(
                    qp[:, :, ca:cb, 1], rec2[:, :, ca:cb, 1],
                    opv[:, :, ca:cb, 1], OP.mult
                )
                nc.vector.tensor_tensor(
                    den[:, :, ca:cb], rec2[:, :, ca:cb, 0],
                    rec2[:, :, ca:cb, 1], OP.add
                )
                den_f = den[:, :, ca:cb].rearrange("p k c g -> p k (c g)")
                nc.vector.reciprocal_approx_fast(den_f, den_f)
                nc.vector.tensor_tensor(
                    qp[:, :, ca:cb, 0], rec2[:, :, ca:cb, 0],
                    opv[:, :, ca:cb, 0], OP.mult
                )
                nc.gpsimd.tensor_tensor(
                    pred[:, :, ca:cb], qp[:, :, ca:cb, 0], qp[:, :, ca:cb, 1],
                    OP.add
                )
                preds[seg, ca] = (pred, den)

            def back2(seg, ca=0, cb=CS):
                """Prediction tail: ratio (Pool) + Ln (ACT)."""
                pred, den = preds.pop((seg, ca))
                if ca == 0:
                    rr_t = scratch.tile([P, K, CS, G], F32, tag="rr_t")
                    out_t = dbuf.tile([P, CS, K, 2, G], F16, tag="out")
                    tiles[seg, "out"] = (rr_t, out_t)
                rr_t, out_t = tiles[seg, "out"]
                nc.gpsimd.tensor_tensor(
                    rr_t[:, :, ca:cb], pred[:, :, ca:cb], den[:, :, ca:cb],
                    OP.mult
                )
                ov = out_t[:].rearrange("p c k u g -> p k c u g")
                nc.scalar.activation(ov[:, :, ca:cb, 1], rr_t[:, :, ca:cb], AF.Ln)
                nc.scalar.activation(
                    ov[:, :, ca:cb, 0], rr_t[:, :, ca:cb], AF.Ln,
                    scale=-1.0, bias=1.0
                )
                fin[seg, ca] = out_t

            def finalize(seg, ca=0, cb=CS):
                out_t = fin.pop((seg, ca))
                s0 = seg * SEG
                nc.sync.dma_start(
                    out_d[:, s0 + ca * K : s0 + cb * K], out_t[:, ca:cb]
                )

            # ---- software pipeline, 2 segments deep ----
            # Per iteration the Pool stream is [qp1/den/pred(s-1) | join(s) |
            # serial(s)]: every stage is data-ready when the in-order queue
            # reaches it, so Pool work spreads across the whole iteration.
            phase_a(0, nsplit=4)
            phase_a(1)
            for seg in range(NSEG):
                if seg >= 1:
                    back(seg - 1)
                A = front(seg)
                a_norm(seg, A)
                if seg >= 1:
                    back2(seg - 1)
                serial(seg, A)
                if seg >= 1:
                    finalize(seg - 1)
                if seg + 2 < NSEG:
                    phase_a(seg + 2)
            hc = CS // 2
            back(NSEG - 1, 0, hc)
            back2(NSEG - 1, 0, hc)
            back(NSEG - 1, hc, CS)
            finalize(NSEG - 1, 0, hc)
            back2(NSEG - 1, hc, CS)
            finalize(NSEG - 1, hc, CS)

    return nc


# ------------------------------------------------------------------
# Host-side full-problem wrapper
# ------------------------------------------------------------------

_B, _T, _K, _SEG = 16384, 500, 10, 100
_G = _B // (P * N_CORES)   # 16 groups per core

_cached = {}


def _build():
    if "nc" not in _cached:
        nc = bacc.Bacc(None, target_bir_lowering=False)
        emit_bkt(nc, G=_G, T=_T, K=_K, SEG=_SEG)
        nc.compile()
        _cached["nc"] = nc
    return _cached["nc"]


def _shard(arr, core):
    """(B,...) -> this core's (P, ..., G) permuted view, seq = g*128 + p."""
    rows = arr[core * P * _G : (core + 1) * P * _G]
    r = rows.reshape(_G, P, *arr.shape[1:])
    order = (1,) + tuple(range(2, r.ndim)) + (0,)
    return np.ascontiguousarray(r.transpose(order))


def kernel(corr, kc, problem, dynamics_logits_table, obs_logits_kc,
           obs_logits_problem, fastbkt_n):
    from concourse.bass_utils import run_bass_kernel_spmd

    corr = np.asarray(corr, dtype=np.float32)
    kc = np.asarray(kc).astype(np.int64)
    problem = np.asarray(problem).astype(np.int64)
    dyn_table = np.asarray(dynamics_logits_table, dtype=np.float32)
    obs_kc = np.asarray(obs_logits_kc, dtype=np.float32)
    obs_prob = np.asarray(obs_logits_problem, dtype=np.float32)

    B, T = corr.shape
    assert B == _B and T == _T, (B, T)

    # host gathers + sign-flip (traffic-neutral input marshaling)
    lls = obs_kc[kc][:, None, :] + obs_prob[problem]       # (B, T, 2)
    sgn = (corr * 2.0 - 1.0).astype(np.float32)            # (B, T)
    zpk = np.empty((B, T, 2), np.float16)
    zpk[:, :, 0] = sgn * lls[:, :, 0]
    zpk[:, :, 1] = -sgn * lls[:, :, 1]
    dyn = dyn_table[kc]                                    # (B, 3)

    nc = _build()
    in_maps = []
    for core in range(N_CORES):
        in_maps.append({
            "zpk": _shard(zpk, core),
            "dyn": _shard(dyn, core),
        })

    res = run_bass_kernel_spmd(
        nc, in_maps, core_ids=list(range(N_CORES)), **_cached.get("run_kwargs", {})
    )
    _cached["last_results"] = res

    # unshard + slot swap: device slot1 = log P(observed), slot0 = log P(other)
    # device time order within a segment is (c, k): t = seg*SEG + c*K + k
    NSEG, CS = _T // _SEG, _SEG // _K
    dev = np.empty((B, T, 2), np.float32)
    for core in range(N_CORES):
        o = res.results[core]["out"].astype(np.float32)    # (P, T, 2, G)
        o = o.reshape(P, NSEG, CS, _K, 2, _G)
        rows = o.transpose(5, 0, 1, 2, 3, 4).reshape(P * _G, T, 2)
        dev[core * P * _G : (core + 1) * P * _G] = rows
    c1 = corr > 0.5
    out = np.empty((B, T, 2), np.float32)
    out[:, :, 1] = np.where(c1, dev[:, :, 1], dev[:, :, 0])
    out[:, :, 0] = np.where(c1, dev[:, :, 0], dev[:, :, 1])
    return out


# revision 35
# speedup vs baseline: 1.0281x; 1.0122x over previous
"""BKT (Bayesian Knowledge Tracing) forward pass on Trainium2, 8 NeuronCores.

The reference's chunked 32-trajectory scan is a 2-state HMM forward pass.
Per (sequence, t):  alpha' = alpha @ (diag(o_t) @ Tr), with o_s(t) =
P(obs_t | L=s) and Tr the 2x2 BKT transition matrix. The output is the
log-softmax over [P(incorrect), P(correct)], i.e. per-t it only depends on
the normalized alpha — every intermediate may carry an arbitrary per-t scale,
which this kernel exploits aggressively.

v3 design (engine-balanced, fp16 2x-mode core, software-pipelined 2 deep):
  - Host sends sign-flipped logits zpk (fp16) so one ACT Sigmoid call gives
    o_s(t) = P(observed outcome | s); the device emits [log(1-r), log r] with
    r = P(observed)/den and the host swaps slots where corr==0 (marshaling).
  - Per-step matrices W = o x (2*Tr) in fp16. Chunk products = two half-chunk
    products of 5 (fp16, range-safe [2^-15, 2^4]), joined in f32 on Pool.
  - Chunk matrices are sum-normalized (one DVE reciprocal per segment), which
    keeps the 50-step f32 serial chunk recursion on Pool bounded (max drift
    2^79 on this data) with NO in-loop renormalization or division.
  - Within-chunk recovery in fp16 from reciprocal-normalized chunk starts,
    restarting mid-chunk (per-chunk/per-half scales cancel in r).
  - Predictions: qp in f32 (DVE), pair-sums on Pool, three ACT Ln calls,
    final log-softmax subtractions on DVE in fp16.
  - Pipeline skew: segment s's fold (DVE) overlaps segment s-1's back half;
    Pool stream ordered [join(s) | preds(s-1) | serial(s)] so the in-order
    Pool queue never blocks on a not-yet-ready stage.
All hot-loop DVE traffic is 2-byte packed (0.52 ns/elem 2x mode); DRAM
arrays are host-packed so every DMA descriptor is a 6.4KB contiguous run.

Sharding: pure data-parallel over batch (2048 sequences/core as 128
partitions x 16 groups); parameter tables gathered on host.
"""

import numpy as np

import concourse.bass as bass
import concourse.bacc as bacc
import concourse.tile as tile
import concourse.mybir as mybir

F32 = mybir.dt.float32
F16 = mybir.dt.float16
AF = mybir.ActivationFunctionType
OP = mybir.AluOpType

P = 128          # partitions
N_CORES = 8
GAMMA = 2.0      # per-step scale baked into Tr: keeps fp16 products ~1


def emit_bkt(nc, G, T, K, SEG):
    """Emit the BKT kernel for one core. Sequences = P*G, free layout (.., g).

    DRAM tensors:
      zpk:  (P, T, 2, G) f16  sign-flipped [guess, slip] logits:
            zpk[..0] = (2c-1)*lg, zpk[..1] = -(2c-1)*ls
      dyn:  (P, 3, G) f32     [logit_pL, logit_pF, logit_pI0]
      out:  (P, T, 2, G) f16  [log(1-r), log r], r = P(observed outcome)
    """
    assert T % SEG == 0 and SEG % K == 0 and K % 2 == 0
    NSEG = T // SEG
    CS = SEG // K          # chunks per segment
    CT = T // K            # total chunks
    H = K // 2             # half-chunk length
    C2 = 2 * CS            # half-chunks per segment

    zpk_d = nc.dram_tensor("zpk", [P, T, 2, G], F16, kind="ExternalInput")
    dyn_d = nc.dram_tensor("dyn", [P, 3, G], F32, kind="ExternalInput")
    out_d = nc.dram_tensor("out", [P, T, 2, G], F16, kind="ExternalOutput")

    with tile.TileContext(nc) as tc:
        with (
            tc.tile_pool(name="singles", bufs=1) as singles,
            tc.tile_pool(name="dbuf", bufs=2) as dbuf,
            tc.tile_pool(name="scratch", bufs=1) as scratch,
        ):
            # ---- per-sequence constants ----
            dyn_t = singles.tile([P, 3, G], F32)
            nc.sync.dma_start(dyn_t[:], dyn_d[:])
            Ttmp = singles.tile([P, 2, 2, G], F32)   # Tr[s][s'][g]
            nc.scalar.activation(Ttmp[:, 0, 0], dyn_t[:, 0], AF.Sigmoid, scale=-1.0)
            nc.scalar.activation(Ttmp[:, 0, 1], dyn_t[:, 0], AF.Sigmoid)
            nc.scalar.activation(Ttmp[:, 1, 0], dyn_t[:, 1], AF.Sigmoid)
            nc.scalar.activation(Ttmp[:, 1, 1], dyn_t[:, 1], AF.Sigmoid, scale=-1.0)
            Tp = singles.tile([P, 2, 2, G], F16)     # gamma * Tr
            nc.scalar.mul(Tp[:], Ttmp[:], GAMMA)

            # chunk-start alphas (f32), all chunks + final carry
            starts = singles.tile([P, CT + 1, 2, G], F32)
            nc.scalar.activation(starts[:, 0, 0], dyn_t[:, 2], AF.Sigmoid, scale=-1.0)
            nc.scalar.activation(starts[:, 0, 1], dyn_t[:, 2], AF.Sigmoid)

            obs = {}       # seg -> op tile (sigmoid outputs)
            mats = {}      # seg -> (Wp, Ah) tiles live into the back half
            tiles = {}     # shared tiles for range-split back phases
            preds = {}     # seg -> (pred, den) awaiting the ratio tail
            fin = {}       # seg -> out tile awaiting store

            def phase_a(seg, nsplit=1):
                s0 = seg * SEG
                zpk = dbuf.tile([P, SEG, 2, G], F16, tag="zpk")
                op_t = dbuf.tile([P, SEG, 2, G], F16, tag="op")
                bounds = [SEG * h // nsplit for h in range(nsplit + 1)]
                for a, b in zip(bounds, bounds[1:]):
                    nc.sync.dma_start(zpk[:, a:b], zpk_d[:, s0 + a : s0 + b])
                    nc.scalar.activation(op_t[:, a:b], zpk[:, a:b], AF.Sigmoid)
                obs[seg] = op_t

            def front(seg):
                """W build + half-chunk fold (DVE) + f32 join (Pool) +
                A-normalization (DVE) staged for the Pool serial chain."""
                # W[t][s][s'][g] = o[t][s][g] * (gamma Tr)[s][s'][g]   (fp16)
                op_t = obs[seg]
                Wp = dbuf.tile([P, SEG, 2, 2, G], F16, tag="Wp")
                nw = 4 if seg == 0 else 1
                wb = [SEG * h // nw for h in range(nw + 1)]
                for a, b in zip(wb, wb[1:]):
                    for s in range(2):   # split keeps reads within 3 AP dims
                        nc.vector.tensor_tensor(
                            Wp[:, a:b, s],
                            op_t[:, a:b, s].unsqueeze(2).broadcast_to(
                                (P, b - a, 2, G)),
                            Tp[:, s].unsqueeze(1).broadcast_to((P, b - a, 2, G)),
                            OP.mult,
                        )
                Wh = Wp[:].rearrange("p (c h) s u g -> p c h s u g", h=H)

                # half-chunk products Ah[c2][i][s'][g] (fp16); step 1 reads
                # W0 x W1 directly ((i,m) split keeps APs legal, no init copy)
                Ah = dbuf.tile([P, C2, 2, 2, G], F16, tag="Ah")
                TMh = dbuf.tile([P, C2, 2, 2, 2, G], F16, tag="TMh")
                for i in range(2):
                    for m in range(2):
                        nc.vector.tensor_tensor(
                            TMh[:, :, i, m],
                            Wh[:, :, 0, i, m].unsqueeze(2).broadcast_to(
                                (P, C2, 2, G)),
                            Wh[:, :, 1, m],
                            OP.mult,
                        )
                nc.vector.tensor_tensor(
                    Ah[:], TMh[:, :, :, 0], TMh[:, :, :, 1], OP.add
                )
                for j in range(2, H):
                    nc.vector.tensor_tensor(
                        TMh[:],
                        Ah[:].unsqueeze(4).broadcast_to((P, C2, 2, 2, 2, G)),
                        Wh[:, :, j].unsqueeze(2).broadcast_to((P, C2, 2, 2, 2, G)),
                        OP.mult,
                    )
                    nc.vector.tensor_tensor(
                        Ah[:], TMh[:, :, :, 0], TMh[:, :, :, 1], OP.add
                    )
                mats[seg] = (Wp, Ah)

                # join halves -> full chunk products A (f32) on Pool
                Ahv = Ah[:].rearrange("p (c h) i u g -> p c h i u g", h=2)
                TM2 = scratch.tile([P, CS, 2, 2, 2, G], F32, tag="TM2")
                for i in range(2):   # split keeps reads within 3 AP dims
                    for m in range(2):
                        nc.vector.tensor_tensor(
                            TM2[:, :, i, m],
                            Ahv[:, :, 0, i, m].unsqueeze(2).broadcast_to(
                                (P, CS, 2, G)),
                            Ahv[:, :, 1, m],
                            OP.mult,
                        )
                A = dbuf.tile([P, CS, 2, 2, G], F32, tag="A")
                nc.vector.tensor_tensor(
                    A[:], TM2[:, :, :, 0], TM2[:, :, :, 1], OP.add
                )
                return A

            def a_norm(seg, A):
                """Sum-normalize chunk matrices (DVE) so the serial chain
                needs no in-loop renorm; any per-chunk scale cancels."""
                uA = scratch.tile([P, CS, 2, G], F32, tag="uA")
                nc.vector.tensor_tensor(uA[:], A[:, :, 0], A[:, :, 1], OP.add)
                tA = scratch.tile([P, CS, G], F32, tag="tA")
                nc.vector.tensor_tensor(tA[:], uA[:, :, 0], uA[:, :, 1], OP.add)
                nc.vector.reciprocal_approx_fast(tA[:], tA[:])
                Af = A[:].rearrange("p c i u g -> p c (i u) g")
                nc.vector.tensor_tensor(
                    Af, Af,
                    tA[:].unsqueeze(2).broadcast_to((P, CS, 4, G)),
                    OP.mult,
                )

            def serial(seg, A):
                """50-step chunk recursion on Pool, f32, no renorm."""
                c0 = seg * CS
                sv = scratch.tile([P, 2, 2, G], F32, tag="sv")
                for cl in range(CS):
                    cg = c0 + cl
                    nc.gpsimd.tensor_tensor(
                        sv[:],
                        starts[:, cg].unsqueeze(2).broadcast_to((P, 2, 2, G)),
                        A[:, cl],
                        OP.mult,
                    )
                    nc.gpsimd.tensor_tensor(
                        starts[:, cg + 1], sv[:, 0], sv[:, 1], OP.add
                    )

            def back(seg, ca=0, cb=CS):
                """Recovery + predictions for chunks [ca, cb) of segment seg.

                rec2 layout (K outermost) keeps every recovery read within
                3 AP dims ((c,m) merge) -> one instruction per step; the
                host undoes the (k, c) interleave when unsharding. The out
                tile is chunk-major so the last segment can drain in halves.
                """
                c0 = seg * CS
                op_t = obs[seg]
                Wp, Ah = mats[seg]
                if cb == CS:
                    obs.pop(seg)
                    mats.pop(seg)
                Wc = Wp[:].rearrange("p (c k) s u g -> p c k s u g", k=K)
                Ahv = Ah[:].rearrange("p (c h) i u g -> p c h i u g", h=2)
                if ca == 0:
                    rec2 = dbuf.tile([P, K, CS, 2, G], F16, tag="rec2")
                    qp = scratch.tile([P, K, CS, 2, G], F32, tag="qp")
                    den = scratch.tile([P, K, CS, G], F32, tag="den")
                    pred = scratch.tile([P, K, CS, G], F32, tag="pred")
                    tiles[seg] = (rec2, qp, den, pred)
                rec2, qp, den, pred = tiles[seg]

                # normalized fp16 chunk starts -> rec2[., 0]; two halves so
                # the first can start before the serial chain finishes
                if ca == 0:
                    ssc = scratch.tile([P, CS, G], F32, tag="ssc")
                    tiles[seg, "ssc"] = ssc
                ssc = tiles[seg, "ssc"]
                nh = max(1, (cb - ca) // (CS // 2))
                sb = [ca + (cb - ca) * h // nh for h in range(nh + 1)]
                for a, b in zip(sb, sb[1:]):
                    stseg = starts[:, c0 + a : c0 + b]
                    n = b - a
                    nc.vector.tensor_tensor(
                        ssc[:, a:b], stseg[:, :, 0], stseg[:, :, 1], OP.add
                    )
                    nc.vector.reciprocal_approx_fast(ssc[:, a:b], ssc[:, a:b])
                    nc.vector.tensor_tensor(
                        rec2[:, 0, a:b], stseg,
                        ssc[:, a:b].unsqueeze(2).broadcast_to((P, n, 2, G)),
                        OP.mult,
                    )

                # mid-chunk restart: S5 = stn16 . Ah_even, renormalized
                nct = cb - ca
                TM5 = scratch.tile([P, CS, 2, 2, G], F16, tag="TM5")
                for i in range(2):
                    nc.vector.tensor_tensor(
                        TM5[:, ca:cb, i],
                        rec2[:, 0, ca:cb, i].unsqueeze(2).broadcast_to(
                            (P, nct, 2, G)),
                        Ahv[:, ca:cb, 0, i],
                        OP.mult,
                    )
                S5 = scratch.tile([P, CS, 2, G], F16, tag="S5")
                nc.vector.tensor_tensor(
                    S5[:, ca:cb], TM5[:, ca:cb, 0], TM5[:, ca:cb, 1], OP.add
                )
                ss5 = scratch.tile([P, CS, G], F32, tag="ss5")
                nc.vector.tensor_tensor(
                    ss5[:, ca:cb], S5[:, ca:cb, 0], S5[:, ca:cb, 1], OP.add
                )
                nc.vector.reciprocal_approx_fast(ss5[:, ca:cb], ss5[:, ca:cb])
                nc.vector.tensor_tensor(
                    rec2[:, H, ca:cb], S5[:, ca:cb],
                    ss5[:, ca:cb].unsqueeze(2).broadcast_to((P, nct, 2, G)),
                    OP.mult,
                )

                # within-chunk recovery (fp16), both halves, 2 instrs/step
                RR = scratch.tile([P, CS, 2, 2, G], F16, tag="RR")
                for j in list(range(1, H)) + list(range(H + 1, K)):
                    nc.vector.tensor_tensor(
                        RR[:, ca:cb],
                        rec2[:, j - 1, ca:cb].unsqueeze(3).broadcast_to(
                            (P, nct, 2, 2, G)),
                        Wc[:, ca:cb, j - 1],
                        OP.mult,
                    )
                    nc.vector.tensor_tensor(
                        rec2[:, j, ca:cb], RR[:, ca:cb, 0], RR[:, ca:cb, 1],
                        OP.add
                    )

                # predictions: den + reciprocal stay on DVE (no cross-
                # engine wait); qp1/pred/ratio on Pool; Ln on ACT.
                opv = op_t[:].rearrange("p (c k) s g -> p k c s g", k=K)
                nc.gpsimd.tensor_tensor(
                    qp[:, :, ca:cb, 1], rec2[:, :, ca:cb, 1],
                    opv[:, :, ca:cb, 1], OP.mult
                )
                nc.vector.tensor_tensor(
                    den[:, :, ca:cb], rec2[:, :, ca:cb, 0],
                    rec2[:, :, ca:cb, 1], OP.add
                )
                den_f = den[:, :, ca:cb].rearrange("p k c g -> p k (c g)")
                nc.vector.reciprocal_approx_fast(den_f, den_f)
                nc.vector.tensor_tensor(
                    qp[:, :, ca:cb, 0], rec2[:, :, ca:cb, 0],
                    opv[:, :, ca:cb, 0], OP.mult
                )
                nc.gpsimd.tensor_tensor(
                    pred[:, :, ca:cb], qp[:, :, ca:cb, 0], qp[:, :, ca:cb, 1],
                    OP.add
                )
                preds[seg, ca] = (pred, den)

            def back2(seg, ca=0, cb=CS):
                """Prediction tail: ratio (Pool) + Ln (ACT)."""
                pred, den = preds.pop((seg, ca))
                if ca == 0:
                    rr_t = scratch.tile([P, K, CS, G], F32, tag="rr_t")
                    out_t = dbuf.tile([P, CS, K, 2, G], F16, tag="out")
                    tiles[seg, "out"] = (rr_t, out_t)
                rr_t, out_t = tiles[seg, "out"]
                nc.gpsimd.tensor_tensor(
                    rr_t[:, :, ca:cb], pred[:, :, ca:cb], den[:, :, ca:cb],
                    OP.mult
                )
                ov = out_t[:].rearrange("p c k u g -> p k c u g")
                nc.scalar.activation(ov[:, :, ca:cb, 1], rr_t[:, :, ca:cb], AF.Ln)
                nc.scalar.activation(
                    ov[:, :, ca:cb, 0], rr_t[:, :, ca:cb], AF.Ln,
                    scale=-1.0, bias=1.0
                )
                fin[seg, ca] = out_t

            def finalize(seg, ca=0, cb=CS):
                out_t = fin.pop((seg, ca))
                s0 = seg * SEG
                nc.sync.dma_start(
                    out_d[:, s0 + ca * K : s0 + cb * K], out_t[:, ca:cb]
                )

            # ---- software pipeline, 2 segments deep ----
            # Per iteration the Pool stream is [qp1/den/pred(s-1) | join(s) |
            # serial(s)]: every stage is data-ready when the in-order queue
            # reaches it, so Pool work spreads across the whole iteration.
            phase_a(0, nsplit=4)
            phase_a(1)
            for seg in range(NSEG):
                if seg >= 1:
                    back(seg - 1)
                A = front(seg)
                a_norm(seg, A)
                if seg >= 1:
                    back2(seg - 1)
                serial(seg, A)
                if seg >= 1:
                    finalize(seg - 1)
                if seg + 2 < NSEG:
                    phase_a(seg + 2)
            hc = CS // 2
            back(NSEG - 1, 0, hc)
            back2(NSEG - 1, 0, hc)
            back(NSEG - 1, hc, CS)
            finalize(NSEG - 1, 0, hc)
            back2(NSEG - 1, hc, CS)
            finalize(NSEG - 1, hc, CS)

    return nc


# ------------------------------------------------------------------
# Host-side full-problem wrapper
# ------------------------------------------------------------------

_B, _T, _K, _SEG = 16384, 500, 10, 100
_G = _B // (P * N_CORES)   # 16 groups per core

_cached = {}


def _build():
    if "nc" not in _cached:
        nc = bacc.Bacc(None, target_bir_lowering=False)
        emit_bkt(nc, G=_G, T=_T, K=_K, SEG=_SEG)
        nc.compile()
        _cached["nc"] = nc
    return _cached["nc"]


def _shard(arr, core):
    """(B,...) -> this core's (P, ..., G) permuted view, seq = g*128 + p."""
    rows = arr[core * P * _G : (core + 1) * P * _G]
    r = rows.reshape(_G, P, *arr.shape[1:])
    order = (1,) + tuple(range(2, r.ndim)) + (0,)
    return np.ascontiguousarray(r.transpose(order))


def kernel(corr, kc, problem, dynamics_logits_table, obs_logits_kc,
           obs_logits_problem, fastbkt_n):
    from concourse.bass_utils import run_bass_kernel_spmd

    corr = np.asarray(corr, dtype=np.float32)
    kc = np.asarray(kc).astype(np.int64)
    problem = np.asarray(problem).astype(np.int64)
    dyn_table = np.asarray(dynamics_logits_table, dtype=np.float32)
    obs_kc = np.asarray(obs_logits_kc, dtype=np.float32)
    obs_prob = np.asarray(obs_logits_problem, dtype=np.float32)

    B, T = corr.shape
    assert B == _B and T == _T, (B, T)

    # host gathers + sign-flip (traffic-neutral input marshaling)
    lls = obs_kc[kc][:, None, :] + obs_prob[problem]       # (B, T, 2)
    sgn = (corr * 2.0 - 1.0).astype(np.float32)            # (B, T)
    zpk = np.empty((B, T, 2), np.float16)
    zpk[:, :, 0] = sgn * lls[:, :, 0]
    zpk[:, :, 1] = -sgn * lls[:, :, 1]
    dyn = dyn_table[kc]                                    # (B, 3)

    nc = _build()
    in_maps = []
    for core in range(N_CORES):
        in_maps.append({
            "zpk": _shard(zpk, core),
            "dyn": _shard(dyn, core),
        })

    res = run_bass_kernel_spmd(
        nc, in_maps, core_ids=list(range(N_CORES)), **_cached.get("run_kwargs", {})
    )
    _cached["last_results"] = res

    # unshard + slot swap: device slot1 = log P(observed), slot0 = log P(other)
    # device time order within a segment is (c, k): t = seg*SEG + c*K + k
    NSEG, CS = _T // _SEG, _SEG // _K
    dev = np.empty((B, T, 2), np.float32)
    for core in range(N_CORES):
        o = res.results[core]["out"].astype(np.float32)    # (P, T, 2, G)
        o = o.reshape(P, NSEG, CS, _K, 2, _G)
        rows = o.transpose(5, 0, 1, 2, 3, 4).reshape(P * _G, T, 2)
        dev[core * P * _G : (core + 1) * P * _G] = rows
    c1 = corr > 0.5
    out = np.empty((B, T, 2), np.float32)
    out[:, :, 1] = np.where(c1, dev[:, :, 1], dev[:, :, 0])
    out[:, :, 0] = np.where(c1, dev[:, :, 0], dev[:, :, 1])
    return out
